# revision 4
# baseline (speedup 1.0000x reference)
"""MoEXLayer forward on 8 Trainium2 NeuronCores.

Math (reference, eval mode):
  W_rec[e] = W*alpha[e] + beta[e];  mu_w = mean_h(W_rec);  var_w = var_h(W_rec)
  Since alpha/beta are constant over h:
     mu_w[e,d]  = Wbar[d]*alpha[e,d] + beta[e,d],   Wbar = mean_h W
     var_w[e,d] = Vw[d]*alpha[e,d]^2,               Vw   = var_h W   (ddof=0)
  mu  = x @ mu_w.T + mean(bias); sig = sqrt(x^2 @ var_w.T + 1e-8)
  logits = erf(mu / (sqrt2*sig)); top-2 softmax -> router weights w1,w2
  out = sum_k w_k * relu(x @ (W*alpha[e_k]).T + bias)

Device strategy (data parallel over tokens, 512 tokens/core):
  - Router matmuls in fp32 (tiny: [128,1024]x[1024,8]).
  - Top-2 per token via the DVE Max8 sort; one-hots via is_equal vs v1/v2.
  - Per-token selected alpha row built with a tiny matmul over the 8-wide
    expert axis: A_k^T = alpha^T @ onehot_k^T, then xe = x * A_k (so only
    K=2 of the E=8 expert GEMMs are ever computed; no gather/scatter).
  - Main GEMMs in bf16 at N=512 against streamed W^T blocks; bias is added
    by a rank-1 ones x bias matmul into the same PSUM accumulation group;
    relu and the router weight are fused on the Scalar engine using
    relu(w*p) = w*relu(p) (w>0 as a softmax weight).
"""

import numpy as np
from contextlib import ExitStack

import sys

if "/opt/trn_rl_repo" not in sys.path:
    sys.path.insert(0, "/opt/trn_rl_repo")

import ml_dtypes
import concourse.bass as bass
import concourse.tile as tile
from concourse import bacc, mybir
from concourse.bass_utils import run_bass_kernel_spmd

FP32 = mybir.dt.float32
BF16 = mybir.dt.bfloat16
AF = mybir.ActivationFunctionType
ALU = mybir.AluOpType

B, S, D, H, E = 2, 2048, 1024, 4096, 8
NCORES = 8
T = (B * S) // NCORES          # 512 tokens per core
NT = T // 128                  # 4 token tiles per core
DC = D // 128                  # 8 contraction chunks
HC = H // 512                  # 8 output column chunks
TG = T // 256                  # 2 selection token groups (N=256 keeps PE fast)


def _emit(ctx: ExitStack, tc: tile.TileContext, io: dict):
    nc = tc.nc
    xt, wt, alpha = io["xt"], io["wt"], io["alpha"]
    alphat, betat = io["alphat"], io["betat"]
    wbar, vw, mb, biasb = io["wbar"], io["vw"], io["mb"], io["biasb"]
    out = io["out"]

    const = ctx.enter_context(tc.tile_pool(name="const", bufs=1))
    persist = ctx.enter_context(tc.tile_pool(name="persist", bufs=1))

    # ---- small constant/parameter tiles ----
    alphat_sb = const.tile([128, E * DC], FP32, name="alphat_sb")
    betat_sb = const.tile([128, E * DC], FP32, name="betat_sb")
    wbar_sb = const.tile([128, DC], FP32, name="wbar_sb")
    vw_sb = const.tile([128, DC], FP32, name="vw_sb")
    mb_sb = const.tile([128, 1], FP32, name="mb_sb")
    alpha_sb = const.tile([E, D], FP32, name="alpha_sb")
    bias_sb = const.tile([1, H], BF16, name="bias_sb")
    ones_sb = const.tile([1, 128], BF16, name="ones_sb")
    ident_sb = const.tile([128, 128], FP32, name="ident_sb")
    eps_sb = const.tile([128, 1], FP32, name="eps_sb")
    nc.vector.memset(eps_sb[:], 2e-8)

    nc.sync.dma_start(alphat_sb[:], alphat[:])
    nc.sync.dma_start(betat_sb[:], betat[:])
    nc.sync.dma_start(wbar_sb[:], wbar[:])
    nc.sync.dma_start(vw_sb[:], vw[:])
    nc.sync.dma_start(mb_sb[:], mb[:])
    nc.sync.dma_start(alpha_sb[:], alpha[:])
    nc.sync.dma_start(bias_sb[:], biasb[:])
    nc.vector.memset(ones_sb[:], 1.0)
    # identity for PE-transpose: keep ones where (p - f) == 0
    nc.vector.memset(ident_sb[:], 1.0)
    nc.gpsimd.affine_select(
        ident_sb[:], ident_sb[:], pattern=[[-1, 128]], base=0,
        channel_multiplier=1, compare_op=ALU.is_equal, fill=0.0,
    )

    # ---- x^T tiles and x^2 tiles ----
    xt_sb = []
    x2_sb = []
    for c in range(DC):
        t_ = persist.tile([128, T], FP32, name=f"xt{c}", tag=f"xt{c}")
        nc.sync.dma_start(t_[:], xt[128 * c:128 * (c + 1), :])
        xt_sb.append(t_)
    for c in range(DC):
        t_ = persist.tile([128, T], FP32, name=f"x2{c}", tag=f"x2{c}")
        nc.scalar.activation(t_[:], xt_sb[c][:], AF.Square)
        x2_sb.append(t_)

    # ---- per-chunk router weight vectors: mu_w^T, var_w^T [128, 8] ----
    mu_w = []
    var_w = []
    for c in range(DC):
        a_sl = alphat_sb[:, E * c:E * (c + 1)]
        b_sl = betat_sb[:, E * c:E * (c + 1)]
        m_ = persist.tile([128, E], FP32, name=f"muw{c}", tag=f"muw{c}")
        nc.vector.scalar_tensor_tensor(
            m_[:], a_sl, wbar_sb[:, c:c + 1], b_sl, op0=ALU.mult, op1=ALU.add)
        mu_w.append(m_)
        a2 = persist.tile([128, E], FP32, name=f"a2_{c}", tag=f"a2_{c}")
        nc.scalar.activation(a2[:], a_sl, AF.Square)
        v_ = persist.tile([128, E], FP32, name=f"varw{c}", tag=f"varw{c}")
        nc.vector.tensor_scalar_mul(v_[:], a2[:], vw_sb[:, c:c + 1])
        var_w.append(v_)

    # ---- router, stage-wise over the 4 token tiles ----
    marg = []   # mu + mean(bias)      [128, 8] fp32
    sig2 = []   # sqrt(2*var + 2e-8)   [128, 8] fp32
    with tc.tile_pool(name="ps_router", bufs=2, space="PSUM") as ps_router:
        for ti in range(NT):
            sl = slice(128 * ti, 128 * (ti + 1))
            mu_ps = ps_router.tile([128, E], FP32, name=f"mu_ps{ti}", tag="mu")
            for c in range(DC):
                nc.tensor.matmul(mu_ps[:], lhsT=xt_sb[c][:, sl], rhs=mu_w[c][:],
                                 start=(c == 0), stop=(c == DC - 1))
            m_ = persist.tile([128, E], FP32, name=f"marg{ti}", tag=f"marg{ti}")
            nc.vector.tensor_scalar_add(m_[:], mu_ps[:], mb_sb[:, 0:1])
            marg.append(m_)
        for ti in range(NT):
            sl = slice(128 * ti, 128 * (ti + 1))
            va_ps = ps_router.tile([128, E], FP32, name=f"va_ps{ti}", tag="va")
            for c in range(DC):
                nc.tensor.matmul(va_ps[:], lhsT=x2_sb[c][:, sl], rhs=var_w[c][:],
                                 start=(c == 0), stop=(c == DC - 1))
            s_ = persist.tile([128, E], FP32, name=f"sig2{ti}", tag=f"sig2{ti}")
            # sqrt(2*var + 2e-8) = sqrt(2) * sigma
            nc.scalar.activation(s_[:], va_ps[:], AF.Sqrt, bias=eps_sb[:, 0:1],
                                 scale=2.0)
            sig2.append(s_)

        logits = []
        for ti in range(NT):
            r_ = persist.tile([128, E], FP32, name=f"rec{ti}", tag=f"rec{ti}")
            nc.vector.reciprocal(r_[:], sig2[ti][:])
            a_ = persist.tile([128, E], FP32, name=f"arg{ti}", tag=f"arg{ti}")
            nc.vector.tensor_tensor(a_[:], marg[ti][:], r_[:], op=ALU.mult)
            logits.append(a_)
        for ti in range(NT):
            nc.scalar.activation(logits[ti][:], logits[ti][:], AF.Erf)

        # top-2, one-hots, softmax weights
        oh = [[None] * NT, [None] * NT]
        wts = []
        for ti in range(NT):
            mx = persist.tile([128, 8], FP32, name=f"mx{ti}", tag=f"mx{ti}")
            nc.vector.max(mx[:], logits[ti][:])
            o1 = persist.tile([128, E], FP32, name=f"oh1_{ti}", tag=f"oh1_{ti}")
            nc.vector.tensor_scalar(o1[:], logits[ti][:], mx[:, 0:1], None,
                                    op0=ALU.is_equal)
            o2 = persist.tile([128, E], FP32, name=f"oh2_{ti}", tag=f"oh2_{ti}")
            nc.vector.tensor_scalar(o2[:], logits[ti][:], mx[:, 1:2], None,
                                    op0=ALU.is_equal)
            oh[0][ti], oh[1][ti] = o1, o2
            d_ = persist.tile([128, 1], FP32, name=f"d21_{ti}", tag=f"d21_{ti}")
            nc.vector.tensor_tensor(d_[:], mx[:, 0:1], mx[:, 1:2], op=ALU.subtract)
            wts.append(d_)
        w_all = []
        for ti in range(NT):
            w_ = persist.tile([128, 2], FP32, name=f"w{ti}", tag=f"w{ti}")
            nc.scalar.activation(w_[:, 0:1], wts[ti][:], AF.Sigmoid)
            nc.vector.tensor_scalar(w_[:, 1:2], w_[:, 0:1], -1.0, 1.0,
                                    op0=ALU.mult, op1=ALU.add)
            w_all.append(w_)

        # transpose one-hots to [E, T] for the selection matmul
        ohT = []
        for k in range(2):
            t_ = persist.tile([E, T], FP32, name=f"ohT{k}", tag=f"ohT{k}")
            ohT.append(t_)
        for k in range(2):
            for ti in range(NT):
                tp = ps_router.tile([E, 128], FP32, name=f"ohTp{k}_{ti}", tag="ohTp")
                nc.tensor.transpose(tp[:], oh[k][ti][:], ident_sb[:])
                nc.vector.tensor_copy(ohT[k][:, 128 * ti:128 * (ti + 1)], tp[:])

    # ---- selection: xe[k] = x * alpha[e_k(t)]  (bf16, [d,t] layout) ----
    xe = [[None] * DC, [None] * DC]
    for k in range(2):
        for c in range(DC):
            xe[k][c] = persist.tile([128, T], BF16, name=f"xe{k}_{c}",
                                    tag=f"xe{k}_{c}")
    with tc.tile_pool(name="ps_sel", bufs=2, space="PSUM") as ps_sel:
        for g in range(TG):
            gsl = slice(256 * g, 256 * (g + 1))
            for k in range(2):
                for c in range(DC):
                    ap_ = ps_sel.tile([128, 256], FP32, name=f"a_ps{g}{k}{c}",
                                      tag="a_ps")
                    nc.tensor.matmul(ap_[:], lhsT=alpha_sb[:, 128 * c:128 * (c + 1)],
                                     rhs=ohT[k][:, gsl], start=True, stop=True)
                    nc.vector.tensor_tensor(xe[k][c][:, gsl], xt_sb[c][:, gsl],
                                            ap_[:], op=ALU.mult)

    # ---- main GEMMs: stream W^T blocks, 2 experts per token tile ----
    wt_pool = ctx.enter_context(tc.tile_pool(name="wts", bufs=1))
    sbuf_out = ctx.enter_context(tc.tile_pool(name="sbuf_out", bufs=1))
    with tc.tile_pool(name="ps_main", bufs=4, space="PSUM") as ps_main:
        for j in range(HC):
            jsl = slice(512 * j, 512 * (j + 1))
            wtiles = []
            for c in range(DC):
                w_ = wt_pool.tile([128, 512], BF16, name=f"wt{c}_{j}",
                                  tag=f"wt{c}", bufs=3)
                nc.sync.dma_start(w_[:], wt[128 * c:128 * (c + 1), jsl])
                wtiles.append(w_)
            for ti in range(NT):
                tsl = slice(128 * ti, 128 * (ti + 1))
                s_tiles = []
                for k in range(2):
                    ps = ps_main.tile([128, 512], FP32, name=f"ps{j}_{ti}_{k}",
                                      tag="ps_main")
                    for c in range(DC):
                        nc.tensor.matmul(ps[:], lhsT=xe[k][c][:, tsl],
                                         rhs=wtiles[c][:],
                                         start=(c == 0), stop=False)
                    nc.tensor.matmul(ps[:], lhsT=ones_sb[:], rhs=bias_sb[:, jsl],
                                     start=False, stop=True)
                    s_ = sbuf_out.tile([128, 512], FP32, name=f"s{j}_{ti}_{k}",
                                       tag=f"s{k}", bufs=3)
                    # relu(w_k * (pre+bias)) == w_k * relu(pre+bias), w_k > 0
                    nc.scalar.activation(s_[:], ps[:], AF.Relu,
                                         scale=w_all[ti][:, k:k + 1])
                    s_tiles.append(s_)
                o_ = sbuf_out.tile([128, 512], FP32, name=f"o{j}_{ti}",
                                   tag="otile", bufs=3)
                nc.vector.tensor_tensor(o_[:], s_tiles[0][:], s_tiles[1][:],
                                        op=ALU.add)
                nc.sync.dma_start(out[tsl, jsl], o_[:])


_CACHE = {}


def _build():
    if "nc" in _CACHE:
        return _CACHE["nc"]
    nc = bacc.Bacc("TRN2", target_bir_lowering=False, debug=False,
                   num_devices=NCORES)
    io = {
        "xt": nc.dram_tensor("xt", [D, T], FP32, kind="ExternalInput").ap(),
        "wt": nc.dram_tensor("wt", [D, H], BF16, kind="ExternalInput").ap(),
        "alpha": nc.dram_tensor("alpha", [E, D], FP32, kind="ExternalInput").ap(),
        "alphat": nc.dram_tensor("alphat", [128, E * DC], FP32,
                                 kind="ExternalInput").ap(),
        "betat": nc.dram_tensor("betat", [128, E * DC], FP32,
                                kind="ExternalInput").ap(),
        "wbar": nc.dram_tensor("wbar", [128, DC], FP32, kind="ExternalInput").ap(),
        "vw": nc.dram_tensor("vw", [128, DC], FP32, kind="ExternalInput").ap(),
        "mb": nc.dram_tensor("mb", [128, 1], FP32, kind="ExternalInput").ap(),
        "biasb": nc.dram_tensor("biasb", [1, H], BF16, kind="ExternalInput").ap(),
        "out": nc.dram_tensor("out", [T, H], FP32, kind="ExternalOutput").ap(),
    }
    with tile.TileContext(nc) as tc, ExitStack() as ctx:
        _emit(ctx, tc, io)
    nc.compile()
    _CACHE["nc"] = nc
    return nc


def _chunk_cols(m):
    # [D, n] -> [128, DC*n] where columns [n*c : n*(c+1)] hold rows 128c..128c+127
    n = m.shape[1]
    return np.ascontiguousarray(
        m.reshape(DC, 128, n).transpose(1, 0, 2).reshape(128, DC * n))


def make_in_maps(x, W, bias, alpha, beta):
    tokens = np.ascontiguousarray(x.reshape(B * S, D))
    Wbar = W.mean(axis=0).astype(np.float32)
    Vw = W.var(axis=0).astype(np.float32)
    mb = np.full((128, 1), bias.mean(), dtype=np.float32)
    wt_bf = np.ascontiguousarray(W.T).astype(ml_dtypes.bfloat16)
    alphat = _chunk_cols(np.ascontiguousarray(alpha.T).astype(np.float32))
    betat = _chunk_cols(np.ascontiguousarray(beta.T).astype(np.float32))
    wbar_c = _chunk_cols(Wbar.reshape(D, 1))
    vw_c = _chunk_cols(Vw.reshape(D, 1))
    biasb = bias.reshape(1, H).astype(ml_dtypes.bfloat16)
    common = dict(wt=wt_bf, alpha=np.ascontiguousarray(alpha, np.float32),
                  alphat=alphat, betat=betat, wbar=wbar_c, vw=vw_c, mb=mb,
                  biasb=biasb)
    maps = []
    for m in range(NCORES):
        xs = np.ascontiguousarray(tokens[T * m:T * (m + 1)].T.astype(np.float32))
        maps.append(dict(xt=xs, **common))
    return maps


def run(x, W, bias, alpha, beta, trace=False, **kw):
    nc = _build()
    maps = make_in_maps(x, W, bias, alpha, beta)
    res = run_bass_kernel_spmd(nc, maps, core_ids=list(range(NCORES)),
                               trace=trace, **kw)
    outs = [res.results[m]["out"] for m in range(NCORES)]
    full = np.concatenate(outs, axis=0).reshape(B, S, H).astype(np.float32)
    return full, res


def kernel(x, W, bias, alpha, beta):
    full, _ = run(np.asarray(x), np.asarray(W), np.asarray(bias),
                  np.asarray(alpha), np.asarray(beta))
    return full


# revision 9
# speedup vs baseline: 1.0995x; 1.0995x over previous
"""MoEXLayer forward on 8 Trainium2 NeuronCores.

Math (reference, eval mode):
  W_rec[e] = W*alpha[e] + beta[e];  mu_w = mean_h(W_rec);  var_w = var_h(W_rec)
  Since alpha/beta are constant over h:
     mu_w[e,d]  = Wbar[d]*alpha[e,d] + beta[e,d],   Wbar = mean_h W
     var_w[e,d] = Vw[d]*alpha[e,d]^2,               Vw   = var_h W   (ddof=0)
  mu  = x @ mu_w.T + mean(bias); sig = sqrt(x^2 @ var_w.T + 1e-8)
  logits = erf(mu / (sqrt2*sig)); top-2 softmax -> router weights w1,w2
  out = sum_k w_k * relu(x @ (W*alpha[e_k]).T + bias)

Device strategy (data parallel over tokens, 512 tokens/core):
  - Router matmuls in fp32 (tiny: [128,1024]x[1024,8]).
  - Top-2 per token via the DVE Max8 sort; one-hots via is_equal vs v1/v2.
  - Per-token selected alpha row built with a tiny matmul over the 8-wide
    expert axis: A_k^T = alpha^T @ onehot_k^T, then xe = x * A_k (so only
    K=2 of the E=8 expert GEMMs are ever computed; no gather/scatter).
  - Main GEMMs in bf16 at N=512 against streamed W^T blocks; bias is added
    by a rank-1 ones x bias matmul into the same PSUM accumulation group;
    relu and the router weight are fused on the Scalar engine using
    relu(w*p) = w*relu(p) (w>0 as a softmax weight).
"""

import numpy as np
from contextlib import ExitStack

import sys

if "/opt/trn_rl_repo" not in sys.path:
    sys.path.insert(0, "/opt/trn_rl_repo")

import ml_dtypes
import concourse.bass as bass
import concourse.tile as tile
from concourse import bacc, mybir
from concourse.bass_utils import run_bass_kernel_spmd

FP32 = mybir.dt.float32
BF16 = mybir.dt.bfloat16
AF = mybir.ActivationFunctionType
ALU = mybir.AluOpType

B, S, D, H, E = 2, 2048, 1024, 4096, 8
NCORES = 8
T = (B * S) // NCORES          # 512 tokens per core
NT = T // 128                  # 4 token tiles per core
DC = D // 128                  # 8 contraction chunks
HC = H // 512                  # 8 output column chunks
TG = T // 256                  # 2 selection token groups (N=256 keeps PE fast)


def _emit(ctx: ExitStack, tc: tile.TileContext, io: dict):
    nc = tc.nc
    xt, wt, alpha = io["xt"], io["wt"], io["alpha"]
    alphat, betat = io["alphat"], io["betat"]
    wbar, vw, mb, biasb = io["wbar"], io["vw"], io["mb"], io["biasb"]
    out = io["out"]

    const = ctx.enter_context(tc.tile_pool(name="const", bufs=1))
    persist = ctx.enter_context(tc.tile_pool(name="persist", bufs=1))

    # ---- small constant/parameter tiles ----
    alphat_sb = const.tile([128, E * DC], FP32, name="alphat_sb")
    betat_sb = const.tile([128, E * DC], FP32, name="betat_sb")
    wbar_sb = const.tile([128, DC], FP32, name="wbar_sb")
    vw_sb = const.tile([128, DC], FP32, name="vw_sb")
    mb_sb = const.tile([128, 1], FP32, name="mb_sb")
    alpha_sb = const.tile([E, D], BF16, name="alpha_sb")
    bias_sb = const.tile([1, H], BF16, name="bias_sb")
    ones_sb = const.tile([1, 128], BF16, name="ones_sb")
    ident_sb = const.tile([128, 128], FP32, name="ident_sb")
    eps_sb = const.tile([128, 1], FP32, name="eps_sb")
    nc.vector.memset(eps_sb[:], 2e-8)

    nc.sync.dma_start(alphat_sb[:], alphat[:])
    nc.sync.dma_start(betat_sb[:], betat[:])
    nc.sync.dma_start(wbar_sb[:], wbar[:])
    nc.sync.dma_start(vw_sb[:], vw[:])
    nc.sync.dma_start(mb_sb[:], mb[:])
    nc.sync.dma_start(alpha_sb[:], alpha[:])
    nc.sync.dma_start(bias_sb[:], biasb[:])
    nc.vector.memset(ones_sb[:], 1.0)
    # identity for PE-transpose: keep ones where (p - f) == 0
    nc.vector.memset(ident_sb[:], 1.0)
    nc.gpsimd.affine_select(
        ident_sb[:], ident_sb[:], pattern=[[-1, 128]], base=0,
        channel_multiplier=1, compare_op=ALU.is_equal, fill=0.0,
    )

    # ---- x^T tiles and x^2 tiles ----
    xt_sb = []
    x2_sb = []
    for c in range(DC):
        t_ = persist.tile([128, T], FP32, name=f"xt{c}", tag=f"xt{c}")
        nc.sync.dma_start(t_[:], xt[128 * c:128 * (c + 1), :])
        xt_sb.append(t_)
    for c in range(DC):
        t_ = persist.tile([128, T], FP32, name=f"x2{c}", tag=f"x2{c}")
        nc.scalar.activation(t_[:], xt_sb[c][:], AF.Square)
        x2_sb.append(t_)

    # ---- per-chunk router weight vectors: mu_w^T, var_w^T [128, 8] ----
    mu_w = []
    var_w = []
    for c in range(DC):
        a_sl = alphat_sb[:, E * c:E * (c + 1)]
        b_sl = betat_sb[:, E * c:E * (c + 1)]
        m_ = persist.tile([128, E], FP32, name=f"muw{c}", tag=f"muw{c}")
        nc.vector.scalar_tensor_tensor(
            m_[:], a_sl, wbar_sb[:, c:c + 1], b_sl, op0=ALU.mult, op1=ALU.add)
        mu_w.append(m_)
        a2 = persist.tile([128, E], FP32, name=f"a2_{c}", tag=f"a2_{c}")
        nc.scalar.activation(a2[:], a_sl, AF.Square)
        v_ = persist.tile([128, E], FP32, name=f"varw{c}", tag=f"varw{c}")
        nc.vector.tensor_scalar_mul(v_[:], a2[:], vw_sb[:, c:c + 1])
        var_w.append(v_)

    # ---- router in [e, t] layout: one pass over all 512 tokens ----
    with tc.tile_pool(name="ps_router", bufs=2, space="PSUM") as ps_router:
        muT_ps = ps_router.tile([E, T], FP32, name="muT_ps", tag="muT")
        for c in range(DC):
            nc.tensor.matmul(muT_ps[:], lhsT=mu_w[c][:], rhs=xt_sb[c][:],
                             start=(c == 0), stop=(c == DC - 1))
        margT = persist.tile([E, T], FP32, name="margT", tag="margT")
        nc.vector.tensor_scalar_add(margT[:], muT_ps[:], mb_sb[0:E, 0:1])
        vaT_ps = ps_router.tile([E, T], FP32, name="vaT_ps", tag="vaT")
        for c in range(DC):
            nc.tensor.matmul(vaT_ps[:], lhsT=var_w[c][:], rhs=x2_sb[c][:],
                             start=(c == 0), stop=(c == DC - 1))
        # sqrt(2*var + 2e-8) = sqrt(2) * sigma
        sig2T = persist.tile([E, T], FP32, name="sig2T", tag="sig2T")
        nc.scalar.activation(sig2T[:], vaT_ps[:], AF.Sqrt, bias=eps_sb[0:E, 0:1],
                             scale=2.0)
        recT = persist.tile([E, T], FP32, name="recT", tag="recT")
        nc.vector.reciprocal(recT[:], sig2T[:])
        logitsT = persist.tile([E, T], FP32, name="logitsT", tag="logitsT")
        nc.vector.tensor_tensor(logitsT[:], margT[:], recT[:], op=ALU.mult)
        nc.scalar.activation(logitsT[:], logitsT[:], AF.Erf)

        # per token tile: transpose logits back, top-2, one-hots, weights
        oh = [[None] * NT, [None] * NT]
        w_all = []
        for ti in range(NT):
            sl = slice(128 * ti, 128 * (ti + 1))
            lg_ps = ps_router.tile([128, E], FP32, name=f"lg_ps{ti}", tag="lg_ps")
            nc.tensor.transpose(lg_ps[:], logitsT[:, sl], ident_sb[0:E, 0:E])
            lg = persist.tile([128, E], FP32, name=f"lg{ti}", tag=f"lg{ti}")
            nc.vector.tensor_copy(lg[:], lg_ps[:])
            mx = persist.tile([128, 8], FP32, name=f"mx{ti}", tag=f"mx{ti}")
            nc.vector.max(mx[:], lg[:])
            o1 = persist.tile([128, E], FP32, name=f"oh1_{ti}", tag=f"oh1_{ti}")
            nc.vector.tensor_scalar(o1[:], lg[:], mx[:, 0:1], None,
                                    op0=ALU.is_equal)
            o2 = persist.tile([128, E], FP32, name=f"oh2_{ti}", tag=f"oh2_{ti}")
            nc.vector.tensor_scalar(o2[:], lg[:], mx[:, 1:2], None,
                                    op0=ALU.is_equal)
            oh[0][ti], oh[1][ti] = o1, o2
            d_ = persist.tile([128, 1], FP32, name=f"d21_{ti}", tag=f"d21_{ti}")
            nc.vector.tensor_tensor(d_[:], mx[:, 0:1], mx[:, 1:2], op=ALU.subtract)
            w_ = persist.tile([128, 2], FP32, name=f"w{ti}", tag=f"w{ti}")
            nc.scalar.activation(w_[:, 0:1], d_[:], AF.Sigmoid)
            nc.vector.tensor_scalar(w_[:, 1:2], w_[:, 0:1], -1.0, 1.0,
                                    op0=ALU.mult, op1=ALU.add)
            w_all.append(w_)

        # transpose one-hots to [E, 256] per token group (bf16 for selection)
        ohT = [[None] * TG, [None] * TG]
        for k in range(2):
            for g in range(TG):
                ohT[k][g] = persist.tile([E, 256], BF16, name=f"ohT{k}_{g}",
                                         tag=f"ohT{k}_{g}")
        for k in range(2):
            for ti in range(NT):
                tp = ps_router.tile([E, 128], FP32, name=f"ohTp{k}_{ti}", tag="ohTp")
                nc.tensor.transpose(tp[:], oh[k][ti][:], ident_sb[:])
                g, h = ti // 2, ti % 2
                nc.vector.tensor_copy(ohT[k][g][:, 128 * h:128 * (h + 1)], tp[:])

    # ---- selection: xe[k][c][g] = x * alpha[e_k(t)]  (bf16, [d,t] layout) ----
    xe = [[[None] * TG for _ in range(DC)] for _ in range(2)]
    for k in range(2):
        for c in range(DC):
            for g in range(TG):
                xe[k][c][g] = persist.tile([128, 256], BF16, name=f"xe{k}_{c}_{g}",
                                           tag=f"xe{k}_{c}_{g}")
    with tc.tile_pool(name="ps_sel", bufs=2, space="PSUM") as ps_sel:
        for g in range(TG):
            gsl = slice(256 * g, 256 * (g + 1))
            for k in range(2):
                for c in range(DC):
                    ap_ = ps_sel.tile([128, 256], FP32, name=f"a_ps{g}{k}{c}",
                                      tag="a_ps")
                    nc.tensor.matmul(ap_[:], lhsT=alpha_sb[:, 128 * c:128 * (c + 1)],
                                     rhs=ohT[k][g][:], start=True, stop=True)
                    nc.vector.tensor_tensor(xe[k][c][g][:], xt_sb[c][:, gsl],
                                            ap_[:], op=ALU.mult)

    # ---- main GEMMs: stream W^T blocks, 2 experts per token tile ----
    wt_pool = ctx.enter_context(tc.tile_pool(name="wts", bufs=1))
    sbuf_out = ctx.enter_context(tc.tile_pool(name="sbuf_out", bufs=1))
    with tc.tile_pool(name="ps_main", bufs=4, space="PSUM") as ps_main:
        for j in range(HC):
            jsl = slice(512 * j, 512 * (j + 1))
            wtiles = []
            for c in range(DC):
                w_ = wt_pool.tile([128, 512], BF16, name=f"wt{c}_{j}",
                                  tag=f"wt{c}", bufs=3)
                nc.sync.dma_start(w_[:], wt[128 * c:128 * (c + 1), jsl])
                wtiles.append(w_)
            for ti in range(NT):
                tsl = slice(128 * ti, 128 * (ti + 1))
                s_tiles = []
                g, hh = ti // 2, ti % 2
                hsl = slice(128 * hh, 128 * (hh + 1))
                for k in range(2):
                    ps = ps_main.tile([128, 512], FP32, name=f"ps{j}_{ti}_{k}",
                                      tag="ps_main")
                    for c in range(DC):
                        nc.tensor.matmul(ps[:], lhsT=xe[k][c][g][:, hsl],
                                         rhs=wtiles[c][:],
                                         start=(c == 0), stop=False)
                    nc.tensor.matmul(ps[:], lhsT=ones_sb[:], rhs=bias_sb[:, jsl],
                                     start=False, stop=True)
                    s_ = sbuf_out.tile([128, 512], FP32, name=f"s{j}_{ti}_{k}",
                                       tag=f"s{k}", bufs=3)
                    # relu(w_k * (pre+bias)) == w_k * relu(pre+bias), w_k > 0
                    nc.scalar.activation(s_[:], ps[:], AF.Relu,
                                         scale=w_all[ti][:, k:k + 1])
                    s_tiles.append(s_)
                o_ = sbuf_out.tile([128, 512], FP32, name=f"o{j}_{ti}",
                                   tag="otile", bufs=3)
                nc.vector.tensor_tensor(o_[:], s_tiles[0][:], s_tiles[1][:],
                                        op=ALU.add)
                nc.sync.dma_start(out[tsl, jsl], o_[:])


_CACHE = {}


def _build():
    if "nc" in _CACHE:
        return _CACHE["nc"]
    nc = bacc.Bacc("TRN2", target_bir_lowering=False, debug=False,
                   num_devices=NCORES)
    io = {
        "xt": nc.dram_tensor("xt", [D, T], FP32, kind="ExternalInput").ap(),
        "wt": nc.dram_tensor("wt", [D, H], BF16, kind="ExternalInput").ap(),
        "alpha": nc.dram_tensor("alpha", [E, D], BF16, kind="ExternalInput").ap(),
        "alphat": nc.dram_tensor("alphat", [128, E * DC], FP32,
                                 kind="ExternalInput").ap(),
        "betat": nc.dram_tensor("betat", [128, E * DC], FP32,
                                kind="ExternalInput").ap(),
        "wbar": nc.dram_tensor("wbar", [128, DC], FP32, kind="ExternalInput").ap(),
        "vw": nc.dram_tensor("vw", [128, DC], FP32, kind="ExternalInput").ap(),
        "mb": nc.dram_tensor("mb", [128, 1], FP32, kind="ExternalInput").ap(),
        "biasb": nc.dram_tensor("biasb", [1, H], BF16, kind="ExternalInput").ap(),
        "out": nc.dram_tensor("out", [T, H], FP32, kind="ExternalOutput").ap(),
    }
    with tile.TileContext(nc) as tc, ExitStack() as ctx:
        _emit(ctx, tc, io)
    nc.compile()
    _CACHE["nc"] = nc
    return nc


def _chunk_cols(m):
    # [D, n] -> [128, DC*n] where columns [n*c : n*(c+1)] hold rows 128c..128c+127
    n = m.shape[1]
    return np.ascontiguousarray(
        m.reshape(DC, 128, n).transpose(1, 0, 2).reshape(128, DC * n))


def make_in_maps(x, W, bias, alpha, beta):
    tokens = np.ascontiguousarray(x.reshape(B * S, D))
    Wbar = W.mean(axis=0).astype(np.float32)
    Vw = W.var(axis=0).astype(np.float32)
    mb = np.full((128, 1), bias.mean(), dtype=np.float32)
    wt_bf = np.ascontiguousarray(W.T).astype(ml_dtypes.bfloat16)
    alphat = _chunk_cols(np.ascontiguousarray(alpha.T).astype(np.float32))
    betat = _chunk_cols(np.ascontiguousarray(beta.T).astype(np.float32))
    wbar_c = _chunk_cols(Wbar.reshape(D, 1))
    vw_c = _chunk_cols(Vw.reshape(D, 1))
    biasb = bias.reshape(1, H).astype(ml_dtypes.bfloat16)
    common = dict(wt=wt_bf, alpha=np.ascontiguousarray(alpha).astype(ml_dtypes.bfloat16),
                  alphat=alphat, betat=betat, wbar=wbar_c, vw=vw_c, mb=mb,
                  biasb=biasb)
    maps = []
    for m in range(NCORES):
        xs = np.ascontiguousarray(tokens[T * m:T * (m + 1)].T.astype(np.float32))
        maps.append(dict(xt=xs, **common))
    return maps


def run(x, W, bias, alpha, beta, trace=False, **kw):
    nc = _build()
    maps = make_in_maps(x, W, bias, alpha, beta)
    res = run_bass_kernel_spmd(nc, maps, core_ids=list(range(NCORES)),
                               trace=trace, **kw)
    outs = [res.results[m]["out"] for m in range(NCORES)]
    full = np.concatenate(outs, axis=0).reshape(B, S, H).astype(np.float32)
    return full, res


def kernel(x, W, bias, alpha, beta):
    full, _ = run(np.asarray(x), np.asarray(W), np.asarray(bias),
                  np.asarray(alpha), np.asarray(beta))
    return full


# revision 14
# speedup vs baseline: 1.1674x; 1.0617x over previous
"""MoEXLayer forward on 8 Trainium2 NeuronCores.

Math (reference, eval mode):
  W_rec[e] = W*alpha[e] + beta[e];  mu_w = mean_h(W_rec);  var_w = var_h(W_rec)
  Since alpha/beta are constant over h:
     mu_w[e,d]  = Wbar[d]*alpha[e,d] + beta[e,d],   Wbar = mean_h W
     var_w[e,d] = Vw[d]*alpha[e,d]^2,               Vw   = var_h W   (ddof=0)
  mu  = x @ mu_w.T + mean(bias); sig = sqrt(x^2 @ var_w.T + 1e-8)
  logits = erf(mu / (sqrt2*sig)); top-2 softmax -> router weights w1,w2
  out = sum_k w_k * relu(x @ (W*alpha[e_k]).T + bias)

Device strategy (data parallel over tokens, 512 tokens/core):
  - Router matmuls in fp32 (tiny: [128,1024]x[1024,8]).
  - Top-2 per token via the DVE Max8 sort; one-hots via is_equal vs v1/v2.
  - Per-token selected alpha row built with a tiny matmul over the 8-wide
    expert axis: A_k^T = alpha^T @ onehot_k^T, then xe = x * A_k (so only
    K=2 of the E=8 expert GEMMs are ever computed; no gather/scatter).
  - Main GEMMs in bf16 at N=512 against streamed W^T blocks; bias is added
    by a rank-1 ones x bias matmul into the same PSUM accumulation group;
    relu and the router weight are fused on the Scalar engine using
    relu(w*p) = w*relu(p) (w>0 as a softmax weight).
"""

import numpy as np
from contextlib import ExitStack

import os
import sys

if "/opt/trn_rl_repo" not in sys.path:
    sys.path.insert(0, "/opt/trn_rl_repo")

import ml_dtypes
import concourse.bass as bass
import concourse.tile as tile
from concourse import bacc, mybir
from concourse.bass_utils import run_bass_kernel_spmd

FP32 = mybir.dt.float32
BF16 = mybir.dt.bfloat16
AF = mybir.ActivationFunctionType
ALU = mybir.AluOpType

B, S, D, H, E = 2, 2048, 1024, 4096, 8
NCORES = 8
T = (B * S) // NCORES          # 512 tokens per core
NT = T // 128                  # 4 token tiles per core
DC = D // 128                  # 8 contraction chunks
HC = H // 512                  # 8 output column chunks
TG = T // 256                  # 2 selection token groups (N=256 keeps PE fast)


def _emit(ctx: ExitStack, tc: tile.TileContext, io: dict):
    nc = tc.nc
    xt, wt, alpha = io["xt"], io["wt"], io["alpha"]
    alphat, betat = io["alphat"], io["betat"]
    wbar, vw, mb, biasb = io["wbar"], io["vw"], io["mb"], io["biasb"]
    out = io["out"]

    const = ctx.enter_context(tc.tile_pool(name="const", bufs=1))
    persist = ctx.enter_context(tc.tile_pool(name="persist", bufs=1))

    # ---- small constant/parameter tiles ----
    alphat_sb = const.tile([128, E * DC], FP32, name="alphat_sb")
    betat_sb = const.tile([128, E * DC], FP32, name="betat_sb")
    wbar_sb = const.tile([128, DC], FP32, name="wbar_sb")
    vw_sb = const.tile([128, DC], FP32, name="vw_sb")
    mb_sb = const.tile([128, 1], FP32, name="mb_sb")
    alpha_sb = const.tile([E, D], BF16, name="alpha_sb")
    bias_sb = const.tile([1, H], BF16, name="bias_sb")
    ones_sb = const.tile([1, 128], BF16, name="ones_sb")
    ident_sb = const.tile([128, 128], FP32, name="ident_sb")
    eps_sb = const.tile([128, 1], FP32, name="eps_sb")
    nc.vector.memset(eps_sb[:], 2e-8)

    nc.sync.dma_start(alphat_sb[:], alphat[:])
    nc.sync.dma_start(betat_sb[:], betat[:])
    nc.sync.dma_start(wbar_sb[:], wbar[:])
    nc.sync.dma_start(vw_sb[:], vw[:])
    nc.sync.dma_start(mb_sb[:], mb[:])
    nc.sync.dma_start(alpha_sb[:], alpha[:])
    nc.sync.dma_start(bias_sb[:], biasb[:])
    nc.vector.memset(ones_sb[:], 1.0)
    # identity for PE-transpose: keep ones where (p - f) == 0
    nc.vector.memset(ident_sb[:], 1.0)
    nc.gpsimd.affine_select(
        ident_sb[:], ident_sb[:], pattern=[[-1, 128]], base=0,
        channel_multiplier=1, compare_op=ALU.is_equal, fill=0.0,
    )

    # ---- x^T tiles and x^2 tiles ----
    xt_sb = []
    x2_sb = []
    for c in range(DC):
        t_ = persist.tile([128, T], FP32, name=f"xt{c}", tag=f"xt{c}")
        nc.sync.dma_start(t_[:], xt[128 * c:128 * (c + 1), :])
        xt_sb.append(t_)
    for c in range(DC):
        t_ = persist.tile([128, T], FP32, name=f"x2{c}", tag=f"x2{c}")
        nc.scalar.activation(t_[:], xt_sb[c][:], AF.Square)
        x2_sb.append(t_)

    # ---- per-chunk router weight vectors: mu_w^T, var_w^T [128, 8] ----
    mu_w = []
    var_w = []
    for c in range(DC):
        a_sl = alphat_sb[:, E * c:E * (c + 1)]
        b_sl = betat_sb[:, E * c:E * (c + 1)]
        m_ = persist.tile([128, E], FP32, name=f"muw{c}", tag=f"muw{c}")
        nc.vector.scalar_tensor_tensor(
            m_[:], a_sl, wbar_sb[:, c:c + 1], b_sl, op0=ALU.mult, op1=ALU.add)
        mu_w.append(m_)
        a2 = persist.tile([128, E], FP32, name=f"a2_{c}", tag=f"a2_{c}")
        nc.scalar.activation(a2[:], a_sl, AF.Square)
        v_ = persist.tile([128, E], FP32, name=f"varw{c}", tag=f"varw{c}")
        nc.vector.tensor_scalar_mul(v_[:], a2[:], vw_sb[:, c:c + 1])
        var_w.append(v_)

    # ---- router + selection, per 256-token group so phase B starts early ----
    xe = [[[None] * TG for _ in range(DC)] for _ in range(2)]
    for k in range(2):
        for c in range(DC):
            for g in range(TG):
                xe[k][c][g] = persist.tile([128, 256], BF16, name=f"xe{k}_{c}_{g}",
                                           tag=f"xe{k}_{c}_{g}")
    w_all = [None] * NT
    with tc.tile_pool(name="ps_router", bufs=2, space="PSUM") as ps_router, \
         tc.tile_pool(name="ps_sel", bufs=2, space="PSUM") as ps_sel:
        for g in range(TG):
            gsl = slice(256 * g, 256 * (g + 1))
            GT = 256
            muT_ps = ps_router.tile([E, GT], FP32, name=f"muT_ps{g}", tag="muT", bufs=1)
            for c in range(DC):
                nc.tensor.matmul(muT_ps[:], lhsT=mu_w[c][:], rhs=xt_sb[c][:, gsl],
                                 start=(c == 0), stop=(c == DC - 1))
            margT = persist.tile([E, GT], FP32, name=f"margT{g}", tag=f"margT{g}")
            nc.vector.tensor_scalar_add(margT[:], muT_ps[:], mb_sb[0:E, 0:1])
            vaT_ps = ps_router.tile([E, GT], FP32, name=f"vaT_ps{g}", tag="vaT", bufs=1)
            for c in range(DC):
                nc.tensor.matmul(vaT_ps[:], lhsT=var_w[c][:], rhs=x2_sb[c][:, gsl],
                                 start=(c == 0), stop=(c == DC - 1))
            # sqrt(2*var + 2e-8) = sqrt(2) * sigma
            sig2T = persist.tile([E, GT], FP32, name=f"sig2T{g}", tag=f"sig2T{g}")
            nc.scalar.activation(sig2T[:], vaT_ps[:], AF.Sqrt,
                                 bias=eps_sb[0:E, 0:1], scale=2.0)
            recT = persist.tile([E, GT], FP32, name=f"recT{g}", tag=f"recT{g}")
            nc.vector.reciprocal(recT[:], sig2T[:])
            logitsT = persist.tile([E, GT], FP32, name=f"logitsT{g}",
                                   tag=f"logitsT{g}")
            nc.vector.tensor_tensor(logitsT[:], margT[:], recT[:], op=ALU.mult)
            nc.scalar.activation(logitsT[:], logitsT[:], AF.Erf)

            # per token tile: transpose logits back, top-2, one-hots, weights
            oh = [[None, None], [None, None]]
            for h in range(2):
                ti = 2 * g + h
                sl = slice(128 * h, 128 * (h + 1))
                lg_ps = ps_router.tile([128, E], FP32, name=f"lg_ps{ti}",
                                       tag="tp", bufs=2)
                nc.tensor.transpose(lg_ps[:], logitsT[:, sl], ident_sb[0:E, 0:E])
                lg = persist.tile([128, E], FP32, name=f"lg{ti}", tag=f"lg{ti}")
                nc.vector.tensor_copy(lg[:], lg_ps[:])
                mx = persist.tile([128, 8], FP32, name=f"mx{ti}", tag=f"mx{ti}")
                nc.vector.max(mx[:], lg[:])
                o1 = persist.tile([128, E], FP32, name=f"oh1_{ti}", tag=f"oh1_{ti}")
                nc.vector.tensor_scalar(o1[:], lg[:], mx[:, 0:1], None,
                                        op0=ALU.is_equal)
                o2 = persist.tile([128, E], FP32, name=f"oh2_{ti}", tag=f"oh2_{ti}")
                nc.vector.tensor_scalar(o2[:], lg[:], mx[:, 1:2], None,
                                        op0=ALU.is_equal)
                oh[0][h], oh[1][h] = o1, o2
                d_ = persist.tile([128, 1], FP32, name=f"d21_{ti}", tag=f"d21_{ti}")
                nc.vector.tensor_tensor(d_[:], mx[:, 0:1], mx[:, 1:2],
                                        op=ALU.subtract)
                w_ = persist.tile([128, 2], FP32, name=f"w{ti}", tag=f"w{ti}")
                nc.scalar.activation(w_[:, 0:1], d_[:], AF.Sigmoid)
                nc.vector.tensor_scalar(w_[:, 1:2], w_[:, 0:1], -1.0, 1.0,
                                        op0=ALU.mult, op1=ALU.add)
                w_all[ti] = w_

            # transpose one-hots to [E, 256] (bf16), then selection + xe
            for k in range(2):
                ohT = persist.tile([E, 256], BF16, name=f"ohT{k}_{g}",
                                   tag=f"ohT{k}_{g}")
                for h in range(2):
                    tp = ps_router.tile([E, 128], FP32, name=f"ohTp{k}_{g}{h}",
                                        tag="tp", bufs=2)
                    nc.tensor.transpose(tp[:], oh[k][h][:], ident_sb[:])
                    nc.vector.tensor_copy(ohT[:, 128 * h:128 * (h + 1)], tp[:])
                for c in range(DC):
                    ap_ = ps_sel.tile([128, 256], FP32, name=f"a_ps{g}{k}{c}",
                                      tag="a_ps")
                    nc.tensor.matmul(ap_[:], lhsT=alpha_sb[:, 128 * c:128 * (c + 1)],
                                     rhs=ohT[:], start=True, stop=True)
                    nc.vector.tensor_tensor(xe[k][c][g][:], xt_sb[c][:, gsl],
                                            ap_[:], op=ALU.mult)

    # ---- main GEMMs: stream W^T blocks; JQ h-chunks share one weight load ----
    JQ = 4
    wt_pool = ctx.enter_context(tc.tile_pool(name="wts", bufs=1))
    sbuf_out = ctx.enter_context(tc.tile_pool(name="sbuf_out", bufs=1))
    with tc.tile_pool(name="ps_main", bufs=2 * JQ, space="PSUM") as ps_main:
        for jq in range(HC // JQ):
            wtiles = [[None] * JQ for _ in range(DC)]
            for c in range(DC):
                for jj in range(JQ):
                    j = jq * JQ + jj
                    w_ = wt_pool.tile([128, 512], BF16, name=f"wt{c}_{j}",
                                      tag=f"wt{c}", bufs=JQ + 2)
                    nc.sync.dma_start(w_[:], wt[128 * c:128 * (c + 1),
                                                512 * j:512 * (j + 1)])
                    wtiles[c][jj] = w_
            for ti in range(NT):
                tsl = slice(128 * ti, 128 * (ti + 1))
                g, hh = ti // 2, ti % 2
                hsl = slice(128 * hh, 128 * (hh + 1))
                s_tiles = [[None] * JQ, [None] * JQ]
                for k in range(2):
                    ps = [ps_main.tile([128, 512], FP32,
                                       name=f"ps{jq}_{ti}_{k}_{jj}",
                                       tag="ps_main") for jj in range(JQ)]
                    for c in range(DC):
                        for jj in range(JQ):
                            nc.tensor.matmul(ps[jj][:], lhsT=xe[k][c][g][:, hsl],
                                             rhs=wtiles[c][jj][:],
                                             start=(c == 0), stop=False)
                    for jj in range(JQ):
                        j = jq * JQ + jj
                        nc.tensor.matmul(ps[jj][:], lhsT=ones_sb[:],
                                         rhs=bias_sb[:, 512 * j:512 * (j + 1)],
                                         start=False, stop=True)
                    for jj in range(JQ):
                        s_ = sbuf_out.tile([128, 512], FP32,
                                           name=f"s{jq}_{ti}_{k}_{jj}",
                                           tag=f"s{k}", bufs=JQ + 2)
                        # relu(w_k*(pre+bias)) == w_k*relu(pre+bias), w_k > 0
                        nc.scalar.activation(s_[:], ps[jj][:], AF.Relu,
                                             scale=w_all[ti][:, k:k + 1])
                        s_tiles[k][jj] = s_
                for jj in range(JQ):
                    j = jq * JQ + jj
                    o_ = sbuf_out.tile([128, 512], FP32, name=f"o{jq}_{ti}_{jj}",
                                       tag="otile", bufs=JQ + 2)
                    nc.vector.tensor_tensor(o_[:], s_tiles[0][jj][:],
                                            s_tiles[1][jj][:], op=ALU.add)
                    nc.sync.dma_start(out[tsl, 512 * j:512 * (j + 1)], o_[:])


_CACHE = {}

if os.environ.get("BASS_LDW_OPT") == "1":
    import concourse.bass_utils as _bu

    _orig_run_command = _bu.run_command

    def _run_command_ldw(cmd, *a, **kw):
        cmd = ["--enable-ldw-opt=true" if c == "--enable-ldw-opt=false" else c
               for c in cmd]
        return _orig_run_command(cmd, *a, **kw)

    _bu.run_command = _run_command_ldw


def _build():
    if "nc" in _CACHE:
        return _CACHE["nc"]
    nc = bacc.Bacc("TRN2", target_bir_lowering=False, debug=False,
                   num_devices=NCORES)
    io = {
        "xt": nc.dram_tensor("xt", [D, T], FP32, kind="ExternalInput").ap(),
        "wt": nc.dram_tensor("wt", [D, H], BF16, kind="ExternalInput").ap(),
        "alpha": nc.dram_tensor("alpha", [E, D], BF16, kind="ExternalInput").ap(),
        "alphat": nc.dram_tensor("alphat", [128, E * DC], FP32,
                                 kind="ExternalInput").ap(),
        "betat": nc.dram_tensor("betat", [128, E * DC], FP32,
                                kind="ExternalInput").ap(),
        "wbar": nc.dram_tensor("wbar", [128, DC], FP32, kind="ExternalInput").ap(),
        "vw": nc.dram_tensor("vw", [128, DC], FP32, kind="ExternalInput").ap(),
        "mb": nc.dram_tensor("mb", [128, 1], FP32, kind="ExternalInput").ap(),
        "biasb": nc.dram_tensor("biasb", [1, H], BF16, kind="ExternalInput").ap(),
        "out": nc.dram_tensor("out", [T, H], FP32, kind="ExternalOutput").ap(),
    }
    with tile.TileContext(nc) as tc, ExitStack() as ctx:
        _emit(ctx, tc, io)
    nc.compile()
    _CACHE["nc"] = nc
    return nc


def _chunk_cols(m):
    # [D, n] -> [128, DC*n] where columns [n*c : n*(c+1)] hold rows 128c..128c+127
    n = m.shape[1]
    return np.ascontiguousarray(
        m.reshape(DC, 128, n).transpose(1, 0, 2).reshape(128, DC * n))


def make_in_maps(x, W, bias, alpha, beta):
    tokens = np.ascontiguousarray(x.reshape(B * S, D))
    Wbar = W.mean(axis=0).astype(np.float32)
    Vw = W.var(axis=0).astype(np.float32)
    mb = np.full((128, 1), bias.mean(), dtype=np.float32)
    wt_bf = np.ascontiguousarray(W.T).astype(ml_dtypes.bfloat16)
    alphat = _chunk_cols(np.ascontiguousarray(alpha.T).astype(np.float32))
    betat = _chunk_cols(np.ascontiguousarray(beta.T).astype(np.float32))
    wbar_c = _chunk_cols(Wbar.reshape(D, 1))
    vw_c = _chunk_cols(Vw.reshape(D, 1))
    biasb = bias.reshape(1, H).astype(ml_dtypes.bfloat16)
    common = dict(wt=wt_bf, alpha=np.ascontiguousarray(alpha).astype(ml_dtypes.bfloat16),
                  alphat=alphat, betat=betat, wbar=wbar_c, vw=vw_c, mb=mb,
                  biasb=biasb)
    maps = []
    for m in range(NCORES):
        xs = np.ascontiguousarray(tokens[T * m:T * (m + 1)].T.astype(np.float32))
        maps.append(dict(xt=xs, **common))
    return maps


def run(x, W, bias, alpha, beta, trace=False, **kw):
    nc = _build()
    maps = make_in_maps(x, W, bias, alpha, beta)
    res = run_bass_kernel_spmd(nc, maps, core_ids=list(range(NCORES)),
                               trace=trace, **kw)
    outs = [res.results[m]["out"] for m in range(NCORES)]
    full = np.concatenate(outs, axis=0).reshape(B, S, H).astype(np.float32)
    return full, res


def kernel(x, W, bias, alpha, beta):
    full, _ = run(np.asarray(x), np.asarray(W), np.asarray(bias),
                  np.asarray(alpha), np.asarray(beta))
    return full
